# revision 1
# baseline (speedup 1.0000x reference)
"""CantorMultiheadFusion — TRN2 Bass kernel (8 NeuronCores, SPMD).

Self-contained: takes FULL inputs, shards across 8 cores, runs one Bass/Tile
NEFF per core via PJRT, gathers the full output.

Sharding: tensor-parallel over heads (2 heads = one 128-wide column slice of
h per core) for the attention, switching to query-parallel for out_proj via
two 256KB AllToAll exchanges (one per 1024-wide query block; each overlaps
the next block's compute).

Algorithm (dense rewrite of the routed sparse attention, exact math):
    A^T[t, q] = mult(q, t) * exp(h_head[t] . h_head[q] / 8)
where mult(q, t) is the multiplicity of key t in routes[q].  mult is built
on-device by gpsimd local_scatter from host-prepared inverse-routing index
lists (index metadata derived from routes).  fusedT and the softmax
denominator come from one PE matmul stream with stationary [h_chunk | ones];
out = fused @ W_out^T + b_out + x computed on the query-owner core.
"""

import sys

if "/opt/trn_rl_repo" not in sys.path:
    sys.path.insert(0, "/opt/trn_rl_repo")

import numpy as np
import ml_dtypes

import concourse.bass as bass  # noqa: F401  (bass must import before mybir use)
import concourse.mybir as mybir
import concourse.tile as tile
from concourse import bacc
from concourse.bass_utils import run_bass_kernel_spmd
from concourse.masks import make_identity

P = 128
S = 2048
D = 1024
H = 16
HD = 64
NCORES = 8
TC = S // P            # 16 key chunks of 128
QB = 1024              # query block width
NQB = S // QB          # 2
BF16 = mybir.dt.bfloat16
F32 = mybir.dt.float32
I16 = mybir.dt.int16
SCALE = 1.0 / np.sqrt(HD)

_PROGRAM_CACHE = {}


def _host_prep(x, routes, W_in, W_out, b_out):
    x2 = np.asarray(x, dtype=np.float32).reshape(S, D)
    routes = np.asarray(routes).astype(np.int64)
    W_in = np.asarray(W_in, dtype=np.float32)
    W_out = np.asarray(W_out, dtype=np.float32)
    b_out = np.asarray(b_out, dtype=np.float32)

    bf = ml_dtypes.bfloat16
    xT = np.ascontiguousarray(x2.T).astype(bf)              # [D, S]
    WoT = np.ascontiguousarray(W_out.T).astype(bf)          # [D, D]

    # Inverse routing lists with multiplicities, bucketed by (key, q-block).
    q_idx = np.repeat(np.arange(S, dtype=np.int64), routes.shape[1])
    t_idx = routes.reshape(-1)
    key = q_idx * S + t_idx
    uniq, counts = np.unique(key, return_counts=True)
    uq = (uniq // S).astype(np.int64)
    ut = (uniq % S).astype(np.int64)
    order = np.lexsort((uq, ut))
    uq, ut, counts = uq[order], ut[order], counts[order]
    qb_of = uq // QB
    bucket_key = ut * NQB + qb_of
    _, bk_counts = np.unique(bucket_key, return_counts=True)
    L = int(bk_counts.max())
    L = (L + 1) // 2 * 2  # even (local_scatter requirement)

    inv_idx = np.full((S, NQB, L), -1, dtype=np.int16)
    inv_val = np.zeros((S, NQB, L), dtype=bf)
    run_start = np.r_[0, np.flatnonzero(np.diff(bucket_key)) + 1]
    run_id = np.zeros(len(uq), dtype=np.int64)
    run_id[run_start] = 1
    run_id = np.cumsum(run_id) - 1
    slot = np.arange(len(uq)) - run_start[run_id]
    inv_idx[ut, qb_of, slot] = (uq - qb_of * QB).astype(np.int16)
    inv_val[ut, qb_of, slot] = counts.astype(bf)
    inv_idx = np.ascontiguousarray(
        inv_idx.reshape(TC, P, NQB, L).transpose(1, 0, 2, 3))
    inv_val = np.ascontiguousarray(
        inv_val.reshape(TC, P, NQB, L).transpose(1, 0, 2, 3))

    has_bias = bool(np.any(b_out != 0.0))
    b128 = np.tile(b_out[None, :], (P, 1)).astype(np.float32)

    in_maps = []
    for c in range(NCORES):
        dsl = slice(c * P, (c + 1) * P)
        x_res = np.stack([x2[k * QB + c * P: k * QB + (c + 1) * P]
                          for k in range(NQB)])             # [NQB, 128, D]
        m = {
            "xT": xT,
            "WiT": np.ascontiguousarray(W_in[dsl, :].T).astype(bf),
            "WoT": WoT,
            "inv_idx": inv_idx,
            "inv_val": inv_val,
            "x_res": np.ascontiguousarray(x_res),
            "tick": np.zeros((P, 8), np.float32),
        }
        if has_bias:
            m["b128"] = b128
        in_maps.append(m)
    return in_maps, L, has_bias


def _assemble(per_core_y, out_dtype):
    out = np.empty((S, D), dtype=out_dtype)
    for c in range(NCORES):
        for k in range(NQB):
            out[k * QB + c * P: k * QB + (c + 1) * P] = per_core_y[c][k]
    return out[None]


def build_program(L, has_bias, repeat=1):
    nc = bacc.Bacc("TRN2", target_bir_lowering=False, debug=False,
                   num_devices=NCORES)

    xT_d = nc.dram_tensor("xT", [D, S], BF16, kind="ExternalInput")
    WiT_d = nc.dram_tensor("WiT", [D, P], BF16, kind="ExternalInput")
    WoT_d = nc.dram_tensor("WoT", [D, D], BF16, kind="ExternalInput")
    inv_idx_d = nc.dram_tensor("inv_idx", [P, TC, NQB, L], I16,
                               kind="ExternalInput")
    inv_val_d = nc.dram_tensor("inv_val", [P, TC, NQB, L], BF16,
                               kind="ExternalInput")
    x_res_d = nc.dram_tensor("x_res", [NQB, P, D], F32, kind="ExternalInput")
    if has_bias:
        b128_d = nc.dram_tensor("b128", [P, D], F32, kind="ExternalInput")
    else:
        b128_d = None
    y_d = nc.dram_tensor("y", [NQB, P, D], F32, kind="ExternalOutput")
    tick_d = nc.dram_tensor("tick", [P, 8], F32, kind="ExternalInput")
    tock_d = nc.dram_tensor("tock", [P, 8], F32, kind="ExternalOutput")
    cc_send = [nc.dram_tensor(f"cc_send{k}", [NCORES, P, P], BF16)
               for k in range(NQB)]
    cc_recv = [nc.dram_tensor(f"cc_recv{k}", [NCORES, P, P], BF16)
               for k in range(NQB)]

    with tile.TileContext(nc) as tc:
        for _ in range(repeat):
            _emit(nc, tc, L, has_bias, xT_d, WiT_d, WoT_d, inv_idx_d,
                  inv_val_d, x_res_d, b128_d, y_d, tick_d, tock_d,
                  cc_send, cc_recv)
    nc.compile()
    return nc


def _emit(nc, tc, L, has_bias, xT_d, WiT_d, WoT_d, inv_idx_d, inv_val_d,
          x_res_d, b128_d, y_d, tick_d, tock_d, cc_send, cc_recv):
    EC = D // P
    NQC = S // 512
    import contextlib
    ctx = contextlib.ExitStack()
    with ctx:
        persist = ctx.enter_context(tc.tile_pool(name="persist", bufs=1))
        tick_sb = persist.tile([P, 8], F32)
        nc.sync.dma_start(tick_sb[:], tick_d[:])
        nc.sync.dma_start(tock_d[:], tick_sb[:])

        hT_sb = persist.tile([P, S], BF16)
        ident = persist.tile([P, P], BF16)
        make_identity(nc, ident[:])
        h_nat = [persist.tile([P, TC, HD + 1], BF16, name=f"h_nat{h}")
                 for h in range(2)]
        inv_idx_sb = persist.tile([P, TC, NQB, L], I16)
        inv_val_sb = persist.tile([P, TC, NQB, L], BF16)
        wo_sb = persist.tile([P, NCORES, D], BF16)
        xr_sb = persist.tile([P, NQB, D], F32)
        if has_bias:
            bb_sb = persist.tile([P, D], F32)

        # ---- Phase A: hT = (x @ W_in_slice^T)^T, h_nat chunks ----
        with tc.tile_pool(name="phA_w", bufs=1) as phA_w, \
             tc.tile_pool(name="phA_ps", bufs=1, space="PSUM") as phA_ps, \
             tc.tile_pool(name="phA_ps2", bufs=2, space="PSUM") as phA_ps2:
            wi_sb = phA_w.tile([P, EC, P], BF16)
            nc.sync.dma_start(
                wi_sb[:], WiT_d.ap().rearrange("(eo ei) d -> ei eo d", ei=P))
            xt_sub = [[phA_w.tile([P, 512], BF16, name=f"xt{ec}_{qc}")
                       for qc in range(NQC)] for ec in range(EC)]
            for qc in range(NQC):
                for ec in range(EC):
                    nc.sync.dma_start(
                        xt_sub[ec][qc][:],
                        xT_d[ec * P:(ec + 1) * P, qc * 512:(qc + 1) * 512])
            nc.gpsimd.dma_start(inv_idx_sb[:], inv_idx_d[:])
            nc.gpsimd.dma_start(inv_val_sb[:], inv_val_d[:])

            for h in range(2):
                nc.vector.memset(h_nat[h][:, :, HD:], 1.0)
            for qc in range(NQC):
                ps = phA_ps.tile([P, 512], F32, tag="hps")
                for ec in range(EC):
                    nc.tensor.matmul(
                        ps[:], wi_sb[:, ec], xt_sub[ec][qc][:],
                        start=(ec == 0), stop=(ec == EC - 1))
                nc.scalar.activation(
                    hT_sb[:, qc * 512:(qc + 1) * 512], ps[:],
                    mybir.ActivationFunctionType.Copy)
                for t in range(qc * 4, qc * 4 + 4):
                    pt = phA_ps2.tile([P, P], BF16)
                    nc.tensor.transpose(
                        pt[:], hT_sb[:, t * P:(t + 1) * P], ident[:])
                    for h in range(2):
                        nc.vector.tensor_copy(
                            h_nat[h][:, t, :HD], pt[:, h * HD:(h + 1) * HD])

        # ---- Phase B (attention) + Phase C (exchange/out_proj), pipelined --
        with tc.tile_pool(name="phB_sc", bufs=2, space="PSUM") as ps_pool, \
             tc.tile_pool(name="phB_f", bufs=1, space="PSUM") as f_pool, \
             tc.tile_pool(name="phB_sb", bufs=4) as sb_pool, \
             tc.tile_pool(name="phB_m", bufs=4) as m_pool, \
             tc.tile_pool(name="phB_o", bufs=2) as o_pool, \
             tc.tile_pool(name="phC", bufs=2) as phC, \
             tc.tile_pool(name="phC_w", bufs=1) as phC_w:

            ft_tiles = {}

            def emit_ft_load(k):
                ft = phC_w.tile([P, NCORES, P], BF16, name=f"ft{k}")
                nc.sync.dma_start(
                    ft[:], cc_recv[k].ap().rearrange("c p q -> p c q"))
                ft_tiles[k] = ft

            def emit_out_proj(k, jb):
                ft = ft_tiles[k]
                ps = ps_pool.tile([P, QB], F32, tag="scores",
                                  name=f"ops_{k}_{jb}")[:, :512]
                for dc in range(NCORES):
                    nc.tensor.matmul(
                        ps[:], ft[:, dc, :],
                        wo_sb[:, dc, jb * 512:(jb + 1) * 512],
                        start=(dc == 0), stop=(dc == NCORES - 1))
                ot = phC.tile([P, 512], F32, tag="out")
                nc.vector.tensor_add(
                    ot[:], ps[:], xr_sb[:, k, jb * 512:(jb + 1) * 512])
                if has_bias:
                    nc.vector.tensor_add(
                        ot[:], ot[:], bb_sb[:, jb * 512:(jb + 1) * 512])
                nc.sync.dma_start(
                    y_d[k, :, jb * 512:(jb + 1) * 512], ot[:])

            for qb in range(NQB):
                facc = [f_pool.tile([HD + 1, 2, 512], F32, tag=f"facc_{h}",
                                    name=f"facc_{qb}_{h}")
                        for h in range(2)]
                for t in range(TC):
                    if qb == 0 and t == 2:
                        # deferred background loads (needed from phase C on)
                        nc.sync.dma_start(
                            wo_sb[:],
                            WoT_d.ap().rearrange("(dc dp) j -> dp dc j", dp=P))
                        nc.sync.dma_start(
                            xr_sb[:], x_res_d.ap().rearrange("k p d -> p k d"))
                        if has_bias:
                            nc.sync.dma_start(bb_sb[:], b128_d[:])
                    if qb > 0 and t == 5:
                        emit_ft_load(qb - 1)
                    mt = m_pool.tile([P, QB], BF16, tag="mask")
                    nc.gpsimd.local_scatter(
                        mt[:], inv_val_sb[:, t, qb], inv_idx_sb[:, t, qb],
                        channels=P, num_elems=QB, num_idxs=L)
                    for h in range(2):
                        hrows = slice(h * HD, (h + 1) * HD)
                        ps = ps_pool.tile([P, QB], F32, tag="scores")
                        for n in range(QB // 512):
                            nc.tensor.matmul(
                                ps[:, n * 512:(n + 1) * 512],
                                hT_sb[hrows, t * P:(t + 1) * P],
                                hT_sb[hrows,
                                      qb * QB + n * 512: qb * QB + (n + 1) * 512],
                                start=True, stop=True)
                        et = sb_pool.tile([P, QB], BF16, tag="exp")
                        nc.scalar.activation(
                            et[:], ps[:], mybir.ActivationFunctionType.Exp,
                            scale=float(SCALE))
                        at = sb_pool.tile([P, QB], BF16, tag="amask")
                        nc.vector.tensor_mul(at[:], et[:], mt[:])
                        for n in range(QB // 512):
                            nc.tensor.matmul(
                                facc[h][:, n, :],
                                h_nat[h][:, t, :],
                                at[:, n * 512:(n + 1) * 512],
                                start=(t == 0), stop=(t == TC - 1))
                if qb > 0:
                    # previous block's out_proj (PE) overlaps this epilogue
                    emit_out_proj(qb - 1, 0)
                    emit_out_proj(qb - 1, 1)
                # drain facc fast (DVE + ACT in parallel), then normalize
                fcp = [o_pool.tile([HD + 1, QB], F32, tag=f"fcp{h}",
                                   name=f"fcp_{qb}_{h}") for h in range(2)]
                nc.vector.tensor_copy(
                    fcp[0][:], facc[0][:].rearrange("p n q -> p (n q)"))
                nc.scalar.activation(
                    fcp[1][:], facc[1][:].rearrange("p n q -> p (n q)"),
                    mybir.ActivationFunctionType.Copy)
                for h in range(2):
                    den0 = o_pool.tile([1, QB], F32, tag="den0")
                    nc.sync.dma_start(den0[:], fcp[h][HD:, :])
                    rec = o_pool.tile([1, QB], F32, tag="rec")
                    nc.vector.reciprocal_approx_fast(rec[:], den0[:])
                    rbc = o_pool.tile([HD, QB], F32, tag="rbc")
                    nc.gpsimd.partition_broadcast(rbc[:], rec[:])
                    fz = o_pool.tile([HD, QB], BF16, tag="fz")
                    nc.vector.tensor_mul(fz[:], fcp[h][:HD, :], rbc[:])
                    nc.sync.dma_start(
                        cc_send[qb][:, h * HD:(h + 1) * HD, :]
                        .rearrange("j p q -> p j q"),
                        fz[:].rearrange("p (j q) -> p j q", j=NCORES))
                nc.gpsimd.collective_compute(
                    kind="AllToAll",
                    op=mybir.AluOpType.bypass,
                    ins=[cc_send[qb][:]],
                    outs=[cc_recv[qb][:]],
                    replica_groups=[list(range(NCORES))],
                )
            emit_ft_load(NQB - 1)
            emit_out_proj(NQB - 1, 0)
            emit_out_proj(NQB - 1, 1)


def _get_program(L, has_bias):
    key = (L, has_bias)
    if key not in _PROGRAM_CACHE:
        _PROGRAM_CACHE[key] = build_program(L, has_bias)
    return _PROGRAM_CACHE[key]


def kernel(x, routes, W_in, W_out, b_out):
    x = np.asarray(x)
    in_dtype = x.dtype
    in_maps, L, has_bias = _host_prep(x, routes, W_in, W_out, b_out)
    nc = _get_program(L, has_bias)
    res = run_bass_kernel_spmd(nc, in_maps, list(range(NCORES)))
    ys = [res.results[c]["y"] for c in range(NCORES)]
    return _assemble(ys, np.float32).astype(in_dtype, copy=False)

